# revision 13
# baseline (speedup 1.0000x reference)
"""GBST layer (pooling) Trainium2 Bass/Tile kernel — pipelined v2.

Math (per sample, x [512, 8192]):
  y = conv1d(x, W[512,512,5], b, VALID)                    # [512, 8188]
  r[l] = score . y[:, l]                                   # conv'd scores
  For w in {1,2,3}: cand_w = block-mean(y, w); s_w = block-mean(r, w)
  att = softmax over the 3 per-position scores; out[l] = sum_w att_w[l] * cand_w[bw(l)]
  out_ds = pairwise mean of out                            # [512, 4096]

Strategy: 1 sample per NeuronCore (8 cores, batch-parallel).
  - conv as 5 shifted bf16 matmuls per (oc, ic) chunk pair on PE (1280 MMs)
  - y kept fully resident in SBUF as bf16 [4][128, 8208] (zero-padded tail
    reproduces the reference's zero-pad semantics exactly)
  - r computed BROADCAST across all 128 partitions directly on PE: the
    stationary operand is score replicated along columns, so
    pr[m, l] = sum_k score[k] y[k, l] for every partition m. All softmax /
    coefficient math is then natural-order 128-wide elementwise work — no
    transposes, no DRAM round trips.
  - out_ds[:,p] = A[p]*y[:,2p] + B[p]*y[:,2p+1] + D[p]*S3[:,g0(p)] + E[p]*S3[:,g1(p)]
    with S3 = stride-3 running sums of y, g0=floor(2p/3), g1=floor((2p+1)/3):
      A = .5*att1[2p]   + .25*(att2[2p]+att2[2p+1])
      B = .5*att1[2p+1] + .25*(att2[2p]+att2[2p+1])
      D = att3[2p]/6 ; E = att3[2p+1]/6
  - software pipeline: combine tile tp (512 out cols) only needs y/r up to
    col 1024*tp+1025, so it runs on DVE+Pool while PE is still convolving
    later tiles. Vector work is split between DVE (nc.vector) and the Pool
    engine (nc.gpsimd); exps on the Act engine (same act table as the conv
    epilogue's Identity/Copy, so no table reloads).

This walrus build caps semaphore waits per instruction very low, so
_fix_wait_overflow() hoists excess waits onto injected same-engine NOPs
placed immediately before the overflowing instruction (safe: no intervening
same-engine instructions, so producers can't depend on anything between).
"""

import numpy as np
import ml_dtypes

import concourse.bass as bass
import concourse.mybir as mybir
from concourse.tile import TileContext

BF16 = mybir.dt.bfloat16
F32 = mybir.dt.float32
AF = mybir.ActivationFunctionType
ALU = mybir.AluOpType

N_CORES = 8
E, L, KS = 512, 8192, 5
LC = L - KS + 1          # 8188 valid conv outputs
LPAD = 8208              # y/r cols incl. zero tail (reads go up to col 8192)
NT = 16                  # conv tiles of 512 positions
OUTL = L // 2            # 4096
NTP = 8                  # combine tiles of 512 output cols
NG = 342                 # stride-3 blocks touched per combine tile

_BUILT = None


def _sap(tile_ap, col_off, dims):
    """Strided SBUF AP on a pool tile: partition dim + custom free dims."""
    pitch, nparts = tile_ap.ap[0]
    return bass.AP(tile_ap.tensor, tile_ap.offset + col_off, [[pitch, nparts]] + dims)


def _fix_wait_overflow(nc):
    """Split >limit semaphore waits onto injected same-engine NOPs."""
    cnt = 0
    for f in nc.m.functions:
        for b in f.blocks:
            newlist = []
            for inst in b.instructions:
                si = inst.sync_info
                if si is not None and si.on_wait:
                    lim = 1
                    waits = list(si.on_wait)
                    while len(waits) > lim:
                        w = waits.pop(0)
                        nop = mybir.InstNoOp(name=f"wfx-{cnt}")
                        cnt += 1
                        nop.engine = inst.engine
                        nop.sync_info = mybir.SyncInfo(on_wait=[w], on_update=[])
                        newlist.append(nop)
                    if cnt and len(waits) != len(si.on_wait):
                        inst.sync_info = mybir.SyncInfo(
                            on_wait=waits, on_update=list(si.on_update)
                        )
                newlist.append(inst)
            b.instructions[:] = newlist
    return cnt


def _build_bass(fix_waits=True):
    nc = bass.Bass("TRN2", target_bir_lowering=False, num_devices=N_CORES)

    xb = nc.dram_tensor("xb", [E, L], BF16, kind="ExternalInput")
    wsb = nc.dram_tensor("wsb", [128, KS * 4 * 4 * 128], BF16, kind="ExternalInput")
    scb = nc.dram_tensor("scb", [128, 4 * 128], BF16, kind="ExternalInput")
    bis = nc.dram_tensor("bis", [128, 4], F32, kind="ExternalInput")
    out_d = nc.dram_tensor("out", [E, OUTL], F32, kind="ExternalOutput")

    with TileContext(nc) as tc:
        with (
            tc.tile_pool(name="const", bufs=1) as kpool,
            tc.tile_pool(name="ybuf", bufs=1) as ypool,
            tc.tile_pool(name="rbuf", bufs=1) as rpool,
            tc.tile_pool(name="xin", bufs=2) as xpool,
            tc.tile_pool(name="ps", bufs=4, space="PSUM") as pspool,
            tc.tile_pool(name="psr", bufs=1, space="PSUM") as prpool,
            tc.tile_pool(name="ps2", bufs=1, space="PSUM") as s2pool,
            tc.tile_pool(name="ps3", bufs=1, space="PSUM") as s3pool,
            tc.tile_pool(name="cw", bufs=1) as cpool,
            tc.tile_pool(name="ab", bufs=2) as abpool,
            tc.tile_pool(name="ot", bufs=3) as opool,
        ):
            w_sb = kpool.tile([128, KS * 4 * 4 * 128], BF16, tag="w")
            for k in range(KS):  # split across DMA queues: first conv tile
                nc.sync.dma_start(  # waits ~8us on a single 2.6MB transfer
                    out=w_sb[:, k * 2048 : (k + 1) * 2048],
                    in_=wsb[:, k * 2048 : (k + 1) * 2048],
                )
            sc_sb = kpool.tile([128, 4 * 128], BF16, tag="sc")
            nc.sync.dma_start(out=sc_sb[:], in_=scb[:])
            bi_sb = kpool.tile([128, 4], F32, tag="bi")
            nc.sync.dma_start(out=bi_sb[:], in_=bis[:])

            ys = [
                ypool.tile([128, LPAD], BF16, name=f"y{c}", tag=f"y{c}")
                for c in range(4)
            ]
            for c in range(4):
                nc.gpsimd.memset(ys[c][:, LC:LPAD], 0.0)
            rb = rpool.tile([128, LPAD], F32, name="rb", tag="rb")
            nc.gpsimd.memset(rb[:, LC:LPAD], 0.0)

            # ---- conv tile: 20 accumulating MMs per oc chunk ----
            def conv_tile(t):
                n0 = 512 * t
                n = min(512, LC - n0)
                xw = min(516, L - n0)
                xt = xpool.tile([128, 4 * 516], BF16, tag="xt")
                for ic in range(4):
                    nc.sync.dma_start(
                        out=xt[:, ic * 516 : ic * 516 + xw],
                        in_=xb[128 * ic : 128 * (ic + 1), n0 : n0 + xw],
                    )
                for oc in range(4):
                    py = pspool.tile([128, 512], F32, tag="py")
                    first = True
                    for ic in range(4):
                        for k in range(KS):
                            nc.tensor.matmul(
                                py[:, :n],
                                lhsT=w_sb[
                                    :,
                                    ((k * 4 + ic) * 4 + oc) * 128 : ((k * 4 + ic) * 4 + oc + 1) * 128,
                                ],
                                rhs=xt[:, ic * 516 + k : ic * 516 + k + n],
                                start=first,
                                stop=(ic == 3 and k == KS - 1),
                            )
                            first = False
                    nc.scalar.activation(
                        ys[oc][:, n0 : n0 + n], py[:, :n], AF.Identity,
                        bias=bi_sb[:, oc : oc + 1], scale=1.0,
                    )

            # ---- broadcast scores: pr[m, l] = r[l] for all partitions m ----
            def emit_rbc(t):
                n0 = 512 * t
                n = min(512, LC - n0)
                pr = prpool.tile([128, 512], F32, tag="pr")
                for cc in range(4):
                    nc.tensor.matmul(
                        pr[:, :n],
                        lhsT=sc_sb[:, cc * 128 : (cc + 1) * 128],
                        rhs=ys[cc][:, n0 : n0 + n],
                        start=(cc == 0),
                        stop=(cc == 3),
                    )
                nc.scalar.activation(rb[:, n0 : n0 + n], pr[:, :n], AF.Copy)

            # ---- coefficient math + combine for one 512-out-col tile ----
            # The Pool engine only supports plain TensorTensor ops, so all
            # constant factors are absorbed into the Act exponentials via
            # the bias term: exp(x + ln c) = c*exp(x). With den' = 2*den and
            # rec' = 1/den' = rec/2:
            #   A = e1[2p]*rec'[2p] + (e2/2)[p]*(rec'[2p]+rec'[2p+1])
            #   D = (e3/3)[g0]*rec'[2p] ; E = (e3/3)[g1]*rec'[2p+1]
            lncst = kpool.tile([128, 3], F32, tag="lncst")
            nc.gpsimd.memset(lncst[:, 0:1], float(np.log(2.0)))
            nc.gpsimd.memset(lncst[:, 1:2], float(np.log(0.5)))
            nc.gpsimd.memset(lncst[:, 2:3], float(np.log(1.0 / 3.0)))
            LN2 = lncst[:, 0:1]
            LNH = lncst[:, 1:2]
            LN3I = lncst[:, 2:3]

            def combine_tile(P0, PN):
                l0 = 2 * P0
                LN = 2 * PN
                G0 = l0 // 3  # first stride-3 block touched
                NG = (l0 + LN - 1) // 3 - G0 + 1

                # --- PE: block-sum scores via shifted accumulating matmuls.
                # S2r[b] = score.(y[2b]+y[2b+1]); S3r[g] = score.(y[3g..3g+2]).
                # Keeps the Act exps' dependencies PE-only, so Act (which also
                # drains conv PSUM) never blocks behind DVE/Pool backlogs.
                ps2 = s2pool.tile([128, 512], F32, tag="ps2")
                for cc in range(4):
                    for u in range(2):
                        nc.tensor.matmul(
                            ps2[:, :PN],
                            lhsT=sc_sb[:, cc * 128 : (cc + 1) * 128],
                            rhs=_sap(ys[cc], l0 + u, [[2, PN]]),
                            start=(cc == 0 and u == 0),
                            stop=(cc == 3 and u == 1),
                        )
                ps3 = s3pool.tile([128, 344], F32, tag="ps3")
                for cc in range(4):
                    for u in range(3):
                        nc.tensor.matmul(
                            ps3[:, :NG],
                            lhsT=sc_sb[:, cc * 128 : (cc + 1) * 128],
                            rhs=_sap(ys[cc], 3 * G0 + u, [[3, NG]]),
                            start=(cc == 0 and u == 0),
                            stop=(cc == 3 and u == 2),
                        )

                # --- Act: all exps; inputs are rb (own writes) or PSUM (PE).
                # e2/e3 are written EXPANDED to per-l resolution via stride-0
                # input dims (each PSUM value read 2x / 3x), so den is two
                # fully contiguous adds instead of five strided ones.
                e1 = abpool.tile([128, 1024], F32, tag="e1")
                nc.scalar.activation(e1[:, :LN], rb[:, l0 : l0 + LN], AF.Exp)
                e2x = cpool.tile([128, 1024], F32, tag="e2x")
                nc.scalar.activation(
                    e2x[:, :LN],
                    bass.AP(ps2.tensor, ps2.offset, [list(ps2.ap[0]), [1, PN], [0, 2]]),
                    AF.Exp, scale=0.5,
                )
                phi = l0 - 3 * G0
                e3x = cpool.tile([128, 1032], F32, tag="e3x")
                nc.scalar.activation(
                    e3x[:, : 3 * NG],
                    bass.AP(ps3.tensor, ps3.offset, [list(ps3.ap[0]), [1, NG], [0, 3]]),
                    AF.Exp, scale=1.0 / 3.0,
                )
                eq2 = cpool.tile([128, 512], F32, tag="eq2")
                nc.scalar.activation(eq2[:, :PN], ps2[:, :PN], AF.Exp, scale=0.5, bias=LNH)
                et3 = cpool.tile([128, 344], F32, tag="et3")
                nc.scalar.activation(
                    et3[:, :NG], ps3[:, :NG], AF.Exp, scale=1.0 / 3.0, bias=LN3I
                )

                # den = e1 + expand2(e2) + expand3(e3);  rec = 0.5/den via
                # exp(-ln(den)+ln.5) on Act (ln/exp/identity/copy share one
                # act table; DVE's reciprocal is ~8us/op).
                den = cpool.tile([128, 1024], F32, tag="den")
                nc.vector.tensor_add(out=den[:, :LN], in0=e1[:, :LN], in1=e2x[:, :LN])
                nc.vector.tensor_add(
                    out=den[:, :LN], in0=den[:, :LN], in1=_sap(e3x, phi, [[1, LN]])
                )
                lgd = cpool.tile([128, 1024], F32, tag="lgd")
                nc.scalar.activation(lgd[:, :LN], den[:, :LN], AF.Ln)
                rec = abpool.tile([128, 1024], F32, tag="rec")
                nc.scalar.activation(rec[:, :LN], lgd[:, :LN], AF.Exp, scale=-1.0, bias=LNH)

                # t1 = e1*rec' (= att1/2);  A/B = t1[even/odd] + (e2/2)*recsum'
                t1 = cpool.tile([128, 1024], F32, tag="t1")
                nc.gpsimd.tensor_mul(out=t1[:, :LN], in0=e1[:, :LN], in1=rec[:, :LN])
                recsum = cpool.tile([128, 512], F32, tag="rsum")
                nc.gpsimd.tensor_add(
                    out=recsum[:, :PN],
                    in0=_sap(rec, 0, [[2, PN]]),
                    in1=_sap(rec, 1, [[2, PN]]),
                )
                e2r = cpool.tile([128, 512], F32, tag="e2r")
                nc.gpsimd.tensor_mul(
                    out=e2r[:, :PN], in0=eq2[:, :PN], in1=recsum[:, :PN]
                )
                Ac = abpool.tile([128, 512], F32, tag="A")
                nc.vector.tensor_add(
                    out=Ac[:, :PN], in0=_sap(t1, 0, [[2, PN]]), in1=e2r[:, :PN]
                )
                Bc = abpool.tile([128, 512], F32, tag="B")
                nc.vector.tensor_add(
                    out=Bc[:, :PN], in0=_sap(t1, 1, [[2, PN]]), in1=e2r[:, :PN]
                )

                # w3 coefficients, compact per v-group. For p%3 != 1, g0==g1 so
                # D+E collapse to F = (e3/3)[g0]*(rec'[2p]+rec'[2p+1]).
                vinfo = []
                sing = 0
                for v in range(3):
                    nq = (PN - v + 2) // 3
                    d0 = (2 * (P0 + v)) // 3 - G0
                    d1 = (2 * (P0 + v) + 1) // 3 - G0
                    if (P0 + v) % 3 == 1:
                        D1 = abpool.tile([128, 172], F32, tag="D1")
                        E1 = abpool.tile([128, 172], F32, tag="E1")
                        nc.vector.tensor_mul(
                            out=D1[:, :nq],
                            in0=_sap(et3, d0, [[2, nq]]),
                            in1=_sap(rec, 2 * v, [[6, nq]]),
                        )
                        nc.gpsimd.tensor_mul(
                            out=E1[:, :nq],
                            in0=_sap(et3, d1, [[2, nq]]),
                            in1=_sap(rec, 2 * v + 1, [[6, nq]]),
                        )
                        vinfo.append((v, nq, d0, d1, D1, E1))
                    else:
                        rs = cpool.tile([128, 172], F32, tag=f"rs{sing}")
                        Fv = abpool.tile([128, 172], F32, tag=f"F{sing}")
                        eng = nc.gpsimd if sing == 0 else nc.vector
                        eng.tensor_add(
                            out=rs[:, :nq],
                            in0=_sap(rec, 2 * v, [[6, nq]]),
                            in1=_sap(rec, 2 * v + 1, [[6, nq]]),
                        )
                        eng.tensor_mul(
                            out=Fv[:, :nq], in0=_sap(et3, d0, [[2, nq]]), in1=rs[:, :nq]
                        )
                        sing += 1
                        vinfo.append((v, nq, d0, d1, Fv, None))

                # combine per 128-channel chunk; split across DVE / Pool
                for cc in range(4):
                    V = nc.vector if cc % 2 == 0 else nc.gpsimd
                    yt = ys[cc]
                    s3y = opool.tile([128, 344], F32, tag="s3y")
                    V.tensor_add(
                        out=s3y[:, :NG],
                        in0=_sap(yt, 3 * G0, [[3, NG]]),
                        in1=_sap(yt, 3 * G0 + 1, [[3, NG]]),
                    )
                    V.tensor_add(
                        out=s3y[:, :NG], in0=s3y[:, :NG],
                        in1=_sap(yt, 3 * G0 + 2, [[3, NG]]),
                    )
                    ot = opool.tile([128, 512], F32, tag="ot")
                    tb = opool.tile([128, 512], F32, tag="tb")
                    V.tensor_mul(
                        out=ot[:, :PN], in0=_sap(yt, l0, [[2, PN]]), in1=Ac[:, :PN]
                    )
                    V.tensor_mul(
                        out=tb[:, :PN], in0=_sap(yt, l0 + 1, [[2, PN]]), in1=Bc[:, :PN]
                    )
                    V.tensor_add(out=ot[:, :PN], in0=ot[:, :PN], in1=tb[:, :PN])
                    for v, nq, d0, d1, Ca, Cb in vinfo:
                        td = opool.tile([128, 172], F32, tag="td")
                        V.tensor_mul(
                            out=td[:, :nq],
                            in0=_sap(s3y, d0, [[2, nq]]),
                            in1=Ca[:, :nq],
                        )
                        if Cb is not None:
                            te = opool.tile([128, 172], F32, tag="te")
                            V.tensor_mul(
                                out=te[:, :nq],
                                in0=_sap(s3y, d1, [[2, nq]]),
                                in1=Cb[:, :nq],
                            )
                            V.tensor_add(
                                out=td[:, :nq], in0=td[:, :nq], in1=te[:, :nq]
                            )
                        V.tensor_add(
                            out=_sap(ot, v, [[3, nq]]),
                            in0=_sap(ot, v, [[3, nq]]),
                            in1=td[:, :nq],
                        )
                    nc.sync.dma_start(
                        out=out_d[128 * cc : 128 * (cc + 1), P0 : P0 + PN],
                        in_=ot[:, :PN],
                    )

            # ---- pipelined schedule ----
            # combine(tp) is ready right after conv(2tp+2)+rbc(2tp+1). The
            # final 512 output cols are split in half: the first half only
            # needs conv(14)/rbc(14), so its vector work overlaps conv(15).
            for t in range(NT - 1):
                conv_tile(t)
                if t >= 1:
                    emit_rbc(t - 1)
                if t in (1, 2):
                    combine_tile(256 * (t - 1), 256)
                elif t >= 4 and t % 2 == 0:
                    combine_tile(256 * (t - 2), 512)
            emit_rbc(NT - 2)
            combine_tile(3584, 256)
            conv_tile(NT - 1)
            emit_rbc(NT - 1)
            combine_tile(3840, 256)

    if fix_waits:
        _fix_wait_overflow(nc)
    return nc


def _prep_inputs(x, conv_w, conv_b, score_w):
    """Per-core input maps. Core b processes sample b."""
    bf = ml_dtypes.bfloat16
    wT = np.ascontiguousarray(conv_w.transpose(1, 0, 2))  # [in, out, k]
    wsb = np.empty((128, KS * 4 * 4 * 128), dtype=bf)
    for k in range(KS):
        for ic in range(4):
            for oc in range(4):
                off = ((k * 4 + ic) * 4 + oc) * 128
                wsb[:, off : off + 128] = wT[
                    128 * ic : 128 * (ic + 1), 128 * oc : 128 * (oc + 1), k
                ].astype(bf)
    # score broadcast along columns: scb[k, cc*128+m] = score[cc*128+k]
    scb = np.empty((128, 4 * 128), dtype=bf)
    for cc in range(4):
        scb[:, cc * 128 : (cc + 1) * 128] = (
            score_w[cc * 128 : (cc + 1) * 128].astype(bf)[:, None]
        )
    bis = np.ascontiguousarray(conv_b.reshape(4, 128).T.astype(np.float32))
    maps = []
    for b in range(N_CORES):
        maps.append({"xb": x[b].astype(bf), "wsb": wsb, "scb": scb, "bis": bis})
    return maps


def kernel(x, conv_w, conv_b, score_w):
    global _BUILT
    from concourse.bass_utils import run_bass_kernel_spmd

    if _BUILT is None:
        _BUILT = _build_bass()
    nc = _BUILT
    x = np.asarray(x, dtype=np.float32)
    maps = _prep_inputs(
        x,
        np.asarray(conv_w, dtype=np.float32),
        np.asarray(conv_b, dtype=np.float32),
        np.asarray(score_w, dtype=np.float32),
    )
    res = run_bass_kernel_spmd(nc, maps, core_ids=list(range(N_CORES)))
    out = np.stack([r["out"] for r in res.results], axis=0)
    return out.astype(np.float32)


# revision 14
# speedup vs baseline: 1.0004x; 1.0004x over previous
"""GBST layer (pooling) Trainium2 Bass/Tile kernel — fully pipelined.

Math (per sample, x [512, 8192]):
  y = conv1d(x, W[512,512,5], b, VALID)                    # [512, 8188]
  r[l] = score . y[:, l]                                   # conv'd scores
  For w in {1,2,3}: cand_w = block-mean(y, w); s_w = block-mean(r, w)
  att = softmax over the 3 per-position scores; out[l] = sum_w att_w[l] * cand_w[bw(l)]
  out_ds = pairwise mean of out                            # [512, 4096]

Strategy: 1 sample per NeuronCore (8 cores, batch-parallel).
  - conv as 5 shifted bf16 matmuls per (oc, ic) chunk pair on PE (1280 MMs);
    y kept fully resident in SBUF as bf16 [4][128, 8208] (zero-padded tail
    reproduces the reference's zero-pad semantics exactly)
  - scores computed BROADCAST across all 128 partitions directly on PE: the
    stationary operand is score replicated along columns, so
    pr[m, l] = sum_k score[k] y[k, l] for every partition m. All softmax /
    coefficient math is then natural-order 128-wide elementwise work — no
    transposes, no DRAM round trips. The w=2 / w=3 block-sum scores are
    extra shifted accumulating matmuls (S2r[b] = score.(y[2b]+y[2b+1]),
    S3r[g] = score.(y[3g]+y[3g+1]+y[3g+2])), which keeps every Act-engine
    exp dependent only on PE — Act also drains conv PSUM, so it must never
    block behind DVE/Pool backlogs (that stalls the PE via PSUM pressure).
  - softmax + downsample fold into 4 coefficient rows:
    out_ds[:,p] = A[p]*y[:,2p] + B[p]*y[:,2p+1] + D[p]*S3y[g0(p)] + E[p]*S3y[g1(p)]
    with S3y = stride-3 sums of y, g0=floor(2p/3), g1=floor((2p+1)/3):
      A = .5*att1[2p]   + .25*(att2[2p]+att2[2p+1])
      B = .5*att1[2p+1] + .25*(att2[2p]+att2[2p+1])
      D = att3[2p]/6 ; E = att3[2p+1]/6
    For p%3 != 1, g0 == g1 so D/E collapse into one coefficient F = D+E.
    Constant factors are folded into exp biases (exp(x+ln c) = c*exp(x));
    1/den is computed as exp(-ln(den)+ln.5) on Act (DVE reciprocal is
    ~8us per [128,1024] op). exp/ln/identity/copy share one act table, so
    there are no act-table reloads. e2/e3 are expanded to per-position
    resolution via stride-0 Act input dims, making the den sum contiguous.
  - software pipeline: the combine for output cols [512tp, 512tp+512) only
    needs y/r through col 1024tp+1025, so it runs on DVE+Pool (2+2 channel
    chunks each) while PE is still convolving later tiles. The final 512
    output cols are split in half so the first half's vector work overlaps
    the last conv tile.

This walrus build caps semaphore waits per instruction very low, so
_fix_wait_overflow() hoists excess waits onto injected same-engine NOPs
placed immediately before the overflowing instruction (safe: no intervening
same-engine instructions, so producers can't depend on anything between).
"""

import numpy as np
import ml_dtypes

import concourse.bass as bass
import concourse.mybir as mybir
from concourse.tile import TileContext

BF16 = mybir.dt.bfloat16
F32 = mybir.dt.float32
AF = mybir.ActivationFunctionType
ALU = mybir.AluOpType

N_CORES = 8
E, L, KS = 512, 8192, 5
LC = L - KS + 1          # 8188 valid conv outputs
LPAD = 8208              # y/r cols incl. zero tail (reads go up to col 8192)
NT = 16                  # conv tiles of 512 positions
OUTL = L // 2            # 4096
NTP = 8                  # combine tiles of 512 output cols
NG = 342                 # stride-3 blocks touched per combine tile

_BUILT = None


def _sap(tile_ap, col_off, dims):
    """Strided SBUF AP on a pool tile: partition dim + custom free dims."""
    pitch, nparts = tile_ap.ap[0]
    return bass.AP(tile_ap.tensor, tile_ap.offset + col_off, [[pitch, nparts]] + dims)


def _fix_wait_overflow(nc):
    """Split >limit semaphore waits onto injected same-engine NOPs."""
    cnt = 0
    for f in nc.m.functions:
        for b in f.blocks:
            newlist = []
            for inst in b.instructions:
                si = inst.sync_info
                if si is not None and si.on_wait:
                    lim = 1
                    waits = list(si.on_wait)
                    while len(waits) > lim:
                        w = waits.pop(0)
                        nop = mybir.InstNoOp(name=f"wfx-{cnt}")
                        cnt += 1
                        nop.engine = inst.engine
                        nop.sync_info = mybir.SyncInfo(on_wait=[w], on_update=[])
                        newlist.append(nop)
                    if cnt and len(waits) != len(si.on_wait):
                        inst.sync_info = mybir.SyncInfo(
                            on_wait=waits, on_update=list(si.on_update)
                        )
                newlist.append(inst)
            b.instructions[:] = newlist
    return cnt


def _build_bass(fix_waits=True):
    nc = bass.Bass("TRN2", target_bir_lowering=False, num_devices=N_CORES)

    xb = nc.dram_tensor("xb", [E, L], BF16, kind="ExternalInput")
    wsb = nc.dram_tensor("wsb", [128, KS * 4 * 4 * 128], BF16, kind="ExternalInput")
    scb = nc.dram_tensor("scb", [128, 4 * 128], BF16, kind="ExternalInput")
    bis = nc.dram_tensor("bis", [128, 4], F32, kind="ExternalInput")
    out_d = nc.dram_tensor("out", [E, OUTL], F32, kind="ExternalOutput")

    with TileContext(nc) as tc:
        with (
            tc.tile_pool(name="const", bufs=1) as kpool,
            tc.tile_pool(name="ybuf", bufs=1) as ypool,
            tc.tile_pool(name="rbuf", bufs=1) as rpool,
            tc.tile_pool(name="xin", bufs=2) as xpool,
            tc.tile_pool(name="ps", bufs=4, space="PSUM") as pspool,
            tc.tile_pool(name="psr", bufs=1, space="PSUM") as prpool,
            tc.tile_pool(name="ps2", bufs=1, space="PSUM") as s2pool,
            tc.tile_pool(name="ps3", bufs=1, space="PSUM") as s3pool,
            tc.tile_pool(name="cw", bufs=1) as cpool,
            tc.tile_pool(name="ab", bufs=2) as abpool,
            tc.tile_pool(name="ot", bufs=3) as opool,
        ):
            w_sb = kpool.tile([128, KS * 4 * 4 * 128], BF16, tag="w")
            for k in range(KS):  # split across DMA queues: first conv tile
                nc.sync.dma_start(  # waits ~8us on a single 2.6MB transfer
                    out=w_sb[:, k * 2048 : (k + 1) * 2048],
                    in_=wsb[:, k * 2048 : (k + 1) * 2048],
                )
            sc_sb = kpool.tile([128, 4 * 128], BF16, tag="sc")
            nc.sync.dma_start(out=sc_sb[:], in_=scb[:])
            bi_sb = kpool.tile([128, 4], F32, tag="bi")
            nc.sync.dma_start(out=bi_sb[:], in_=bis[:])

            ys = [
                ypool.tile([128, LPAD], BF16, name=f"y{c}", tag=f"y{c}")
                for c in range(4)
            ]
            for c in range(4):
                nc.gpsimd.memset(ys[c][:, LC:LPAD], 0.0)
            rb = rpool.tile([128, LPAD], F32, name="rb", tag="rb")
            nc.gpsimd.memset(rb[:, LC:LPAD], 0.0)

            # ---- conv tile: 20 accumulating MMs per oc chunk ----
            def conv_tile(t):
                n0 = 512 * t
                n = min(512, LC - n0)
                xw = min(516, L - n0)
                xt = xpool.tile([128, 4 * 516], BF16, tag="xt")
                for ic in range(4):
                    nc.sync.dma_start(
                        out=xt[:, ic * 516 : ic * 516 + xw],
                        in_=xb[128 * ic : 128 * (ic + 1), n0 : n0 + xw],
                    )
                for oc in range(4):
                    py = pspool.tile([128, 512], F32, tag="py")
                    first = True
                    for ic in range(4):
                        for k in range(KS):
                            nc.tensor.matmul(
                                py[:, :n],
                                lhsT=w_sb[
                                    :,
                                    ((k * 4 + ic) * 4 + oc) * 128 : ((k * 4 + ic) * 4 + oc + 1) * 128,
                                ],
                                rhs=xt[:, ic * 516 + k : ic * 516 + k + n],
                                start=first,
                                stop=(ic == 3 and k == KS - 1),
                            )
                            first = False
                    nc.scalar.activation(
                        ys[oc][:, n0 : n0 + n], py[:, :n], AF.Identity,
                        bias=bi_sb[:, oc : oc + 1], scale=1.0,
                    )

            # ---- broadcast scores: pr[m, l] = r[l] for all partitions m ----
            def emit_rbc(t):
                n0 = 512 * t
                n = min(512, LC - n0)
                pr = prpool.tile([128, 512], F32, tag="pr")
                for cc in range(4):
                    nc.tensor.matmul(
                        pr[:, :n],
                        lhsT=sc_sb[:, cc * 128 : (cc + 1) * 128],
                        rhs=ys[cc][:, n0 : n0 + n],
                        start=(cc == 0),
                        stop=(cc == 3),
                    )
                nc.scalar.activation(rb[:, n0 : n0 + n], pr[:, :n], AF.Copy)

            # ---- coefficient math + combine for one 512-out-col tile ----
            # The Pool engine only supports plain TensorTensor ops, so all
            # constant factors are absorbed into the Act exponentials via
            # the bias term: exp(x + ln c) = c*exp(x). With den' = 2*den and
            # rec' = 1/den' = rec/2:
            #   A = e1[2p]*rec'[2p] + (e2/2)[p]*(rec'[2p]+rec'[2p+1])
            #   D = (e3/3)[g0]*rec'[2p] ; E = (e3/3)[g1]*rec'[2p+1]
            lncst = kpool.tile([128, 3], F32, tag="lncst")
            nc.gpsimd.memset(lncst[:, 0:1], float(np.log(2.0)))
            nc.gpsimd.memset(lncst[:, 1:2], float(np.log(0.5)))
            nc.gpsimd.memset(lncst[:, 2:3], float(np.log(1.0 / 3.0)))
            LN2 = lncst[:, 0:1]
            LNH = lncst[:, 1:2]
            LN3I = lncst[:, 2:3]

            def combine_tile(P0, PN):
                l0 = 2 * P0
                LN = 2 * PN
                G0 = l0 // 3  # first stride-3 block touched
                NG = (l0 + LN - 1) // 3 - G0 + 1

                # --- PE: block-sum scores via shifted accumulating matmuls.
                # S2r[b] = score.(y[2b]+y[2b+1]); S3r[g] = score.(y[3g..3g+2]).
                # Keeps the Act exps' dependencies PE-only, so Act (which also
                # drains conv PSUM) never blocks behind DVE/Pool backlogs.
                ps2 = s2pool.tile([128, 512], F32, tag="ps2")
                for cc in range(4):
                    for u in range(2):
                        nc.tensor.matmul(
                            ps2[:, :PN],
                            lhsT=sc_sb[:, cc * 128 : (cc + 1) * 128],
                            rhs=_sap(ys[cc], l0 + u, [[2, PN]]),
                            start=(cc == 0 and u == 0),
                            stop=(cc == 3 and u == 1),
                        )
                ps3 = s3pool.tile([128, 344], F32, tag="ps3")
                for cc in range(4):
                    for u in range(3):
                        nc.tensor.matmul(
                            ps3[:, :NG],
                            lhsT=sc_sb[:, cc * 128 : (cc + 1) * 128],
                            rhs=_sap(ys[cc], 3 * G0 + u, [[3, NG]]),
                            start=(cc == 0 and u == 0),
                            stop=(cc == 3 and u == 2),
                        )

                # --- Act: all exps; inputs are rb (own writes) or PSUM (PE).
                # e2/e3 are written EXPANDED to per-l resolution via stride-0
                # input dims (each PSUM value read 2x / 3x), so den is two
                # fully contiguous adds instead of five strided ones.
                e1 = abpool.tile([128, 1024], F32, tag="e1")
                nc.scalar.activation(e1[:, :LN], rb[:, l0 : l0 + LN], AF.Exp)
                e2x = cpool.tile([128, 1024], F32, tag="e2x")
                nc.scalar.activation(
                    e2x[:, :LN],
                    bass.AP(ps2.tensor, ps2.offset, [list(ps2.ap[0]), [1, PN], [0, 2]]),
                    AF.Exp, scale=0.5,
                )
                phi = l0 - 3 * G0
                e3x = cpool.tile([128, 1032], F32, tag="e3x")
                nc.scalar.activation(
                    e3x[:, : 3 * NG],
                    bass.AP(ps3.tensor, ps3.offset, [list(ps3.ap[0]), [1, NG], [0, 3]]),
                    AF.Exp, scale=1.0 / 3.0,
                )
                eq2 = cpool.tile([128, 512], F32, tag="eq2")
                nc.scalar.activation(eq2[:, :PN], ps2[:, :PN], AF.Exp, scale=0.5, bias=LNH)
                et3 = cpool.tile([128, 344], F32, tag="et3")
                nc.scalar.activation(
                    et3[:, :NG], ps3[:, :NG], AF.Exp, scale=1.0 / 3.0, bias=LN3I
                )

                # den = e1 + expand2(e2) + expand3(e3);  rec = 0.5/den via
                # exp(-ln(den)+ln.5) on Act (ln/exp/identity/copy share one
                # act table; DVE's reciprocal is ~8us/op).
                den = cpool.tile([128, 1024], F32, tag="den")
                nc.vector.tensor_add(out=den[:, :LN], in0=e1[:, :LN], in1=e2x[:, :LN])
                nc.vector.tensor_add(
                    out=den[:, :LN], in0=den[:, :LN], in1=_sap(e3x, phi, [[1, LN]])
                )
                lgd = cpool.tile([128, 1024], F32, tag="lgd")
                nc.scalar.activation(lgd[:, :LN], den[:, :LN], AF.Ln)
                rec = abpool.tile([128, 1024], F32, tag="rec")
                nc.scalar.activation(rec[:, :LN], lgd[:, :LN], AF.Exp, scale=-1.0, bias=LNH)

                # t1 = e1*rec' (= att1/2);  A/B = t1[even/odd] + (e2/2)*recsum'
                t1 = cpool.tile([128, 1024], F32, tag="t1")
                nc.gpsimd.tensor_mul(out=t1[:, :LN], in0=e1[:, :LN], in1=rec[:, :LN])
                recsum = cpool.tile([128, 512], F32, tag="rsum")
                nc.gpsimd.tensor_add(
                    out=recsum[:, :PN],
                    in0=_sap(rec, 0, [[2, PN]]),
                    in1=_sap(rec, 1, [[2, PN]]),
                )
                e2r = cpool.tile([128, 512], F32, tag="e2r")
                nc.gpsimd.tensor_mul(
                    out=e2r[:, :PN], in0=eq2[:, :PN], in1=recsum[:, :PN]
                )
                Ac = abpool.tile([128, 512], F32, tag="A")
                nc.vector.tensor_add(
                    out=Ac[:, :PN], in0=_sap(t1, 0, [[2, PN]]), in1=e2r[:, :PN]
                )
                Bc = abpool.tile([128, 512], F32, tag="B")
                nc.vector.tensor_add(
                    out=Bc[:, :PN], in0=_sap(t1, 1, [[2, PN]]), in1=e2r[:, :PN]
                )

                # w3 coefficients, compact per v-group. For p%3 != 1, g0==g1 so
                # D+E collapse to F = (e3/3)[g0]*(rec'[2p]+rec'[2p+1]).
                vinfo = []
                sing = 0
                for v in range(3):
                    nq = (PN - v + 2) // 3
                    d0 = (2 * (P0 + v)) // 3 - G0
                    d1 = (2 * (P0 + v) + 1) // 3 - G0
                    if (P0 + v) % 3 == 1:
                        D1 = abpool.tile([128, 172], F32, tag="D1")
                        E1 = abpool.tile([128, 172], F32, tag="E1")
                        nc.vector.tensor_mul(
                            out=D1[:, :nq],
                            in0=_sap(et3, d0, [[2, nq]]),
                            in1=_sap(rec, 2 * v, [[6, nq]]),
                        )
                        nc.gpsimd.tensor_mul(
                            out=E1[:, :nq],
                            in0=_sap(et3, d1, [[2, nq]]),
                            in1=_sap(rec, 2 * v + 1, [[6, nq]]),
                        )
                        vinfo.append((v, nq, d0, d1, D1, E1))
                    else:
                        rs = cpool.tile([128, 172], F32, tag=f"rs{sing}")
                        Fv = abpool.tile([128, 172], F32, tag=f"F{sing}")
                        eng = nc.gpsimd if sing == 0 else nc.vector
                        eng.tensor_add(
                            out=rs[:, :nq],
                            in0=_sap(rec, 2 * v, [[6, nq]]),
                            in1=_sap(rec, 2 * v + 1, [[6, nq]]),
                        )
                        eng.tensor_mul(
                            out=Fv[:, :nq], in0=_sap(et3, d0, [[2, nq]]), in1=rs[:, :nq]
                        )
                        sing += 1
                        vinfo.append((v, nq, d0, d1, Fv, None))

                # combine per 128-channel chunk; split across DVE / Pool
                for cc in range(4):
                    V = nc.vector if cc % 2 == 0 else nc.gpsimd
                    yt = ys[cc]
                    s3y = opool.tile([128, 344], F32, tag="s3y")
                    V.tensor_add(
                        out=s3y[:, :NG],
                        in0=_sap(yt, 3 * G0, [[3, NG]]),
                        in1=_sap(yt, 3 * G0 + 1, [[3, NG]]),
                    )
                    V.tensor_add(
                        out=s3y[:, :NG], in0=s3y[:, :NG],
                        in1=_sap(yt, 3 * G0 + 2, [[3, NG]]),
                    )
                    ot = opool.tile([128, 512], F32, tag="ot")
                    tb = opool.tile([128, 512], F32, tag="tb")
                    V.tensor_mul(
                        out=ot[:, :PN], in0=_sap(yt, l0, [[2, PN]]), in1=Ac[:, :PN]
                    )
                    V.tensor_mul(
                        out=tb[:, :PN], in0=_sap(yt, l0 + 1, [[2, PN]]), in1=Bc[:, :PN]
                    )
                    V.tensor_add(out=ot[:, :PN], in0=ot[:, :PN], in1=tb[:, :PN])
                    for v, nq, d0, d1, Ca, Cb in vinfo:
                        td = opool.tile([128, 172], F32, tag="td")
                        V.tensor_mul(
                            out=td[:, :nq],
                            in0=_sap(s3y, d0, [[2, nq]]),
                            in1=Ca[:, :nq],
                        )
                        if Cb is not None:
                            te = opool.tile([128, 172], F32, tag="te")
                            V.tensor_mul(
                                out=te[:, :nq],
                                in0=_sap(s3y, d1, [[2, nq]]),
                                in1=Cb[:, :nq],
                            )
                            V.tensor_add(
                                out=td[:, :nq], in0=td[:, :nq], in1=te[:, :nq]
                            )
                        V.tensor_add(
                            out=_sap(ot, v, [[3, nq]]),
                            in0=_sap(ot, v, [[3, nq]]),
                            in1=td[:, :nq],
                        )
                    nc.sync.dma_start(
                        out=out_d[128 * cc : 128 * (cc + 1), P0 : P0 + PN],
                        in_=ot[:, :PN],
                    )

            # ---- pipelined schedule ----
            # combine(tp) is ready right after conv(2tp+2)+rbc(2tp+1). The
            # final 512 output cols are split in half: the first half only
            # needs conv(14)/rbc(14), so its vector work overlaps conv(15).
            for t in range(NT - 1):
                conv_tile(t)
                if t >= 1:
                    emit_rbc(t - 1)
                if t in (1, 2):
                    combine_tile(256 * (t - 1), 256)
                elif t >= 4 and t % 2 == 0:
                    combine_tile(256 * (t - 2), 512)
            emit_rbc(NT - 2)
            combine_tile(3584, 256)
            conv_tile(NT - 1)
            emit_rbc(NT - 1)
            combine_tile(3840, 256)

    if fix_waits:
        _fix_wait_overflow(nc)
    return nc


def _prep_inputs(x, conv_w, conv_b, score_w):
    """Per-core input maps. Core b processes sample b."""
    bf = ml_dtypes.bfloat16
    wT = np.ascontiguousarray(conv_w.transpose(1, 0, 2))  # [in, out, k]
    wsb = np.empty((128, KS * 4 * 4 * 128), dtype=bf)
    for k in range(KS):
        for ic in range(4):
            for oc in range(4):
                off = ((k * 4 + ic) * 4 + oc) * 128
                wsb[:, off : off + 128] = wT[
                    128 * ic : 128 * (ic + 1), 128 * oc : 128 * (oc + 1), k
                ].astype(bf)
    # score broadcast along columns: scb[k, cc*128+m] = score[cc*128+k]
    scb = np.empty((128, 4 * 128), dtype=bf)
    for cc in range(4):
        scb[:, cc * 128 : (cc + 1) * 128] = (
            score_w[cc * 128 : (cc + 1) * 128].astype(bf)[:, None]
        )
    bis = np.ascontiguousarray(conv_b.reshape(4, 128).T.astype(np.float32))
    maps = []
    for b in range(N_CORES):
        maps.append({"xb": x[b].astype(bf), "wsb": wsb, "scb": scb, "bis": bis})
    return maps


def kernel(x, conv_w, conv_b, score_w):
    global _BUILT
    from concourse.bass_utils import run_bass_kernel_spmd

    if _BUILT is None:
        _BUILT = _build_bass()
    nc = _BUILT
    x = np.asarray(x, dtype=np.float32)
    maps = _prep_inputs(
        x,
        np.asarray(conv_w, dtype=np.float32),
        np.asarray(conv_b, dtype=np.float32),
        np.asarray(score_w, dtype=np.float32),
    )
    res = run_bass_kernel_spmd(nc, maps, core_ids=list(range(N_CORES)))
    out = np.stack([r["out"] for r in res.results], axis=0)
    return out.astype(np.float32)
